# revision 1
# baseline (speedup 1.0000x reference)
"""Multi-head attention (B=2, S=2048, D=1024, H=16, dk=64) on 8 trn2 cores.

Sharding: batch (2) x head-group (4 heads each) = 8 shards.
Core c handles batch b = c // 4, heads g = c % 4 (heads 4g..4g+3).

Host-side prep per core:
  - inputs transposed to [d, s] so the contraction dim lands on SBUF
    partitions with no on-chip transposes,
  - Wq/Wk/Wv column-sharded per head group (1/sqrt(dk) folded into Wq/bq),
  - Wo row-sharded, transposed to [c, j],
  - each core emits a transposed partial output [1024, 2048]; host sums the
    4 partials per batch (bo/4 folded into each partial) and transposes back.

On-chip dataflow (per core):
  KT[m,s], QT[m,s] = W.T @ xT          (m = head-major dim, 256)
  V[k, h, dv(+ones)]                    (natural row layout, ones col for denom)
  scoresT[k, q] = KT_h.T @ QT_h         (per head, transposed scores)
  attn_u = exp(scoresT)                 (no max subtraction; scores ~ N(0,1))
  numden[65, q] = V'_h.T @ attn_u       (rows 0-63 numerator, row 64 denom)
  attn_cat[c, q] = numden[:64] * recip(denom)   (recip broadcast across the
                                                 64 partitions via a K=1 PE
                                                 matmul with a hi/lo bf16
                                                 split of the fp32 recip)
  outT[j, q] = woT.T @ attn_cat + bo/4

All matmuls run in bf16 (fp32 PSUM accumulate); fp32r was rejected because
its walrus lowering only supports 1 sync wait per matmul. Full-kernel
relative error vs the fp32 reference is ~6e-3.
"""

from contextlib import ExitStack

import ml_dtypes
import numpy as np

import concourse.bacc as bacc
import concourse.mybir as mybir
import concourse.tile as tile
from concourse.bass_utils import run_bass_kernel_spmd

F32 = mybir.dt.float32
BF16 = mybir.dt.bfloat16

D = 1024          # d_model
S = 2048          # sequence length
HCORE = 4         # heads per core
DK = 64           # head dim
M = HCORE * DK    # 256 sharded projection width
P = 128

N_CORES = 8
ST = 512          # s-tile (free dim of projection / q-tile)
N_ST = S // ST    # 4
N_DT = D // P     # 8 contraction tiles for projections
N_KT = S // P     # 16 k-tiles for attention
N_JT = D // P     # 8 output row tiles
GRP = 2           # score slots per psum group (2 banks, double buffered)




def build_mha_tile(tc, outs, ins):
    nc = tc.nc
    xqT, xkT, xvT = ins["xqT"], ins["xkT"], ins["xvT"]
    wq, wk, wv, woT = ins["wq"], ins["wk"], ins["wv"], ins["woT"]
    bq, bk, bvb, bo4 = ins["bq"], ins["bk"], ins["bvb"], ins["bo4"]
    outT = outs["outT"]

    ctx = ExitStack()
    ec = ctx.enter_context
    const = ec(tc.tile_pool(name="const", bufs=1))
    persist = ec(tc.tile_pool(name="persist", bufs=1))
    xq_pool = ec(tc.tile_pool(name="xq", bufs=8))
    xv_pool = ec(tc.tile_pool(name="xv", bufs=2))
    au_pool = ec(tc.tile_pool(name="au", bufs=12))
    out_pool = ec(tc.tile_pool(name="outb", bufs=4))
    small = ec(tc.tile_pool(name="small", bufs=2))
    rb_pool = ec(tc.tile_pool(name="rb", bufs=2))
    nds_pool = ec(tc.tile_pool(name="nds", bufs=2))
    proj_ps = ec(tc.tile_pool(name="proj_ps", bufs=1, space="PSUM"))
    sc_ps = ec(tc.tile_pool(name="sc_ps", bufs=2, space="PSUM"))
    av_ps = ec(tc.tile_pool(name="av_ps", bufs=3, space="PSUM"))

    # Const loads are interleaved with the x-input streams below so the
    # first score matmuls' inputs (wk+K st0, wq+Q st0) are at the head of
    # the DMA queue instead of behind all 8 weight/bias loads. The `warm`
    # DVE touches absorb each bias tile's DMA-lane wait so it never lands
    # as a 2nd sync wait on a hot DVE op (walrus allows only 1).
    warm = const.tile([P, 16], F32)
    ones_sb = const.tile([1, DK], BF16)
    nc.vector.memset(ones_sb, 1.0)

    # ---- persistent activations ----
    QT_sb = persist.tile([P, 2, S], BF16)          # [p, mt, s]
    KT_sb = persist.tile([P, 2, S], BF16)
    V_sb = persist.tile([P, N_KT, HCORE, DK + 1], BF16)   # [p, kt, h, dv']
    cat_sb = persist.tile([P, 2, S], BF16)         # attn_cat [c, ct, s]

    nc.vector.memset(V_sb[:, :, :, DK], 1.0)      # ones column for denominators

    def project_qk(xT3, w_sb, b_sb, dst_sb, st):
        """dst[m, st-slice] = w.T @ xT + b  for m=256 (2 partition tiles)."""
        xt = xq_pool.tile([P, N_DT, ST], BF16, tag="xt")
        nc.sync.dma_start(xt, xT3[:, :, st * ST:(st + 1) * ST])
        for mt in range(2):
            ps = proj_ps.tile([P, ST], F32, tag="proj", name="qk_ps")
            for dt in range(N_DT):
                nc.tensor.matmul(
                    ps,
                    w_sb[:, dt, mt * P:(mt + 1) * P],
                    xt[:, dt, :],
                    start=(dt == 0), stop=(dt == N_DT - 1))
            nc.vector.tensor_scalar_add(
                dst_sb[:, mt, st * ST:(st + 1) * ST], ps, b_sb[:, mt:mt + 1])

    def project_v():
        """V[k, h, dv] = xvT[:, k].T @ wv + bv, written into V_sb rows."""
        xvT3 = xvT.rearrange("(dt p) s -> p dt s", p=P)
        for ktg in range(N_KT // 4):
            xt = xv_pool.tile([P, N_DT, 4 * P], BF16, tag="xvt")
            nc.sync.dma_start(
                xt, xvT3[:, :, ktg * 4 * P:(ktg + 1) * 4 * P])
            for kl in range(4):
                kt = ktg * 4 + kl
                ps = av_ps.tile([P, ST], F32, tag="av", name="v_ps")[:, :M]
                for dt in range(N_DT):
                    nc.tensor.matmul(
                        ps, xt[:, dt, kl * P:(kl + 1) * P], wv_sb[:, dt, :],
                        start=(dt == 0), stop=(dt == N_DT - 1))
                nc.vector.tensor_add(
                    out=V_sb[:, kt, :, 0:DK],
                    in0=ps.rearrange("p (h d) -> p h d", h=HCORE),
                    in1=bvb_sb.rearrange("p (h d) -> p h d", h=HCORE))

    def attention(qt):
        """scoresT -> exp -> attn@V' -> normalize into cat_sb, for all heads."""
        qs = slice(qt * ST, (qt + 1) * ST)
        for hp in range(2):                       # head pairs (0,1), (2,3)
            heads = (2 * hp, 2 * hp + 1)
            nd = {h: av_ps.tile([P, ST], F32, tag="av", name=f"av_ps_{h}")
                  for h in heads}
            all_slots = [(kt, h) for kt in range(N_KT) for h in heads]
            for g in range(len(all_slots) // GRP):    # groups of GRP slots
                sc = sc_ps.tile([P, GRP, ST], F32, tag="sc")
                au = au_pool.tile([P, GRP, ST], BF16, tag="au")
                slots = all_slots[g * GRP:(g + 1) * GRP]
                for i, (kt, h) in enumerate(slots):
                    mt, p0 = h // 2, (h % 2) * DK
                    nc.tensor.matmul(
                        sc[:, i, :],
                        KT_sb[p0:p0 + DK, mt, kt * P:(kt + 1) * P],
                        QT_sb[p0:p0 + DK, mt, qs],
                        start=True, stop=True)
                nc.scalar.activation(au, sc, mybir.ActivationFunctionType.Exp)
                for i, (kt, h) in enumerate(slots):
                    nc.tensor.matmul(
                        nd[h][:DK + 1, :],
                        V_sb[:, kt, h, :],
                        au[:, i, :],
                        start=(kt == 0), stop=(kt == N_KT - 1))
            for h in heads:
                mt, p0 = h // 2, (h % 2) * DK
                # copy num+denom to SBUF first so the PSUM accumulator bank
                # frees early for the next head pair
                nds = nds_pool.tile([DK + 1, ST], F32, tag="nds")
                nc.vector.tensor_copy(nds, nd[h][:DK + 1, :])
                recip = small.tile([1, ST], F32, tag="recip")
                nc.vector.reciprocal(recip, nds[DK:DK + 1, :])
                # broadcast recip across 64 partitions via a K=1 PE matmul;
                # hi+lo bf16 split keeps ~16 mantissa bits of the fp32 recip
                rhi = small.tile([1, ST], BF16, tag="rhi")
                rlo = small.tile([1, ST], BF16, tag="rlo")
                nc.vector.tensor_copy(rhi, recip)
                nc.vector.tensor_tensor(
                    rlo, recip, rhi, mybir.AluOpType.subtract)
                rb_ps = proj_ps.tile([P, ST], F32, tag="proj",
                                     name="rb_ps")[:DK, :]
                nc.tensor.matmul(rb_ps, ones_sb, rhi, start=True, stop=False)
                nc.tensor.matmul(rb_ps, ones_sb, rlo, start=False, stop=True)
                rb = rb_pool.tile([DK, ST], F32, tag="rb")
                nc.vector.tensor_copy(rb, rb_ps)
                nc.vector.tensor_mul(
                    out=cat_sb[p0:p0 + DK, mt, qs],
                    in0=nds[0:DK, :], in1=rb)

    def out_proj(qt):
        qs = slice(qt * ST, (qt + 1) * ST)
        outT3 = outT.rearrange("(jt p) s -> p jt s", p=P)
        ob = out_pool.tile([P, N_JT, ST], BF16, tag="ob")
        ps_pool = av_ps if qt == N_ST - 1 else proj_ps
        ps_tag = "av" if qt == N_ST - 1 else "proj"
        for jt in range(N_JT):
            ps = ps_pool.tile([P, ST], F32, tag=ps_tag, name="op_ps")
            for ct in range(2):
                nc.tensor.matmul(
                    ps,
                    woT_sb[:, ct, jt * P:(jt + 1) * P],
                    cat_sb[:, ct, qs],
                    start=(ct == 0), stop=(ct == 1))
            nc.vector.tensor_scalar_add(
                ob[:, jt, :], ps, bo4_sb[:, jt:jt + 1])
            if jt == N_JT // 2 - 1:
                nc.sync.dma_start(
                    outT3[:, :N_JT // 2, qs], ob[:, :N_JT // 2, :])
        nc.sync.dma_start(outT3[:, N_JT // 2:, qs], ob[:, N_JT // 2:, :])

    xqT3 = xqT.rearrange("(dt p) s -> p dt s", p=P)
    xkT3 = xkT.rearrange("(dt p) s -> p dt s", p=P)
    wk_sb = const.tile_from(wk)
    bk_sb = const.tile_from(bk)
    nc.vector.tensor_copy(warm[:, 2:4], bk_sb)
    project_qk(xkT3, wk_sb, bk_sb, KT_sb, 0)
    wq_sb = const.tile_from(wq)
    bq_sb = const.tile_from(bq)
    nc.vector.tensor_copy(warm[:, 0:2], bq_sb)
    project_qk(xqT3, wq_sb, bq_sb, QT_sb, 0)
    for st in range(1, N_ST):
        project_qk(xkT3, wk_sb, bk_sb, KT_sb, st)
    wv_sb = const.tile_from(wv)
    bvb_sb = const.tile_from(bvb)
    nc.vector.tensor_copy(warm[:, 12:16], bvb_sb[:, 0:4])
    project_v()
    for st in range(1, N_ST):
        project_qk(xqT3, wq_sb, bq_sb, QT_sb, st)
    woT_sb = const.tile_from(woT)
    bo4_sb = const.tile_from(bo4)
    nc.vector.tensor_copy(warm[:, 4:12], bo4_sb)
    for qt in range(N_ST):
        attention(qt)
        out_proj(qt)
    ctx.close()


def build_bass():
    nc = bacc.Bacc(trn_type="TRN2", target_bir_lowering=False, debug=False)
    ins = {
        "xqT": nc.dram_tensor("xqT", (D, S), BF16, kind="ExternalInput").ap(),
        "xkT": nc.dram_tensor("xkT", (D, S), BF16, kind="ExternalInput").ap(),
        "xvT": nc.dram_tensor("xvT", (D, S), BF16, kind="ExternalInput").ap(),
        "wq": nc.dram_tensor("wq", (P, N_DT, M), BF16, kind="ExternalInput").ap(),
        "wk": nc.dram_tensor("wk", (P, N_DT, M), BF16, kind="ExternalInput").ap(),
        "wv": nc.dram_tensor("wv", (P, N_DT, M), BF16, kind="ExternalInput").ap(),
        "woT": nc.dram_tensor("woT", (P, 2, D), BF16, kind="ExternalInput").ap(),
        "bq": nc.dram_tensor("bq", (P, 2), F32, kind="ExternalInput").ap(),
        "bk": nc.dram_tensor("bk", (P, 2), F32, kind="ExternalInput").ap(),
        "bvb": nc.dram_tensor("bvb", (P, M), F32, kind="ExternalInput").ap(),
        "bo4": nc.dram_tensor("bo4", (P, N_JT), F32, kind="ExternalInput").ap(),
    }
    outs = {
        "outT": nc.dram_tensor("outT", (D, S), BF16, kind="ExternalOutput").ap(),
    }
    with tile.TileContext(nc) as tc:
        build_mha_tile(tc, outs, ins)
    nc.compile()
    return nc


def shard_inputs(query, key, value, Wq, bq, Wk, bk, Wv, bv, Wo, bo):
    """Build the 8 per-core input maps (all host-side numpy layout prep)."""
    def prep_w(W, ms, scale=1.0):
        # [d, m] -> [p, dt, m]
        wT = (np.asarray(W)[ms, :].T * scale).astype(ml_dtypes.bfloat16)
        return np.ascontiguousarray(
            wT.reshape(N_DT, P, M).transpose(1, 0, 2))

    def prep_b(b, ms, scale=1.0):
        return np.ascontiguousarray(
            (np.asarray(b)[ms] * scale).astype(np.float32).reshape(2, P).T)

    in_maps = []
    for c in range(N_CORES):
        b_idx, g = divmod(c, N_CORES // 2)
        ms = slice(g * M, (g + 1) * M)
        woT = np.ascontiguousarray(Wo[:, ms].T.astype(np.float32))
        in_maps.append({
            "xqT": np.ascontiguousarray(query[b_idx].T.astype(ml_dtypes.bfloat16)),
            "xkT": np.ascontiguousarray(key[b_idx].T.astype(ml_dtypes.bfloat16)),
            "xvT": np.ascontiguousarray(value[b_idx].T.astype(ml_dtypes.bfloat16)),
            "wq": prep_w(Wq, ms, 1.0 / np.sqrt(DK)),
            "wk": prep_w(Wk, ms),
            "wv": prep_w(Wv, ms),
            "woT": np.ascontiguousarray(
                woT.astype(ml_dtypes.bfloat16).reshape(2, P, D).transpose(1, 0, 2)),
            "bq": prep_b(bq, ms, 1.0 / np.sqrt(DK)),
            "bk": prep_b(bk, ms),
            "bvb": np.ascontiguousarray(
                np.tile(np.asarray(bv)[ms].astype(np.float32), (P, 1))),
            "bo4": np.ascontiguousarray(
                (np.asarray(bo) / (N_CORES // 2)).astype(np.float32)
                .reshape(N_JT, P).T),
        })
    return in_maps


_NC_CACHE = None
_RUNNER_CACHE = None


def _get_nc():
    global _NC_CACHE
    if _NC_CACHE is None:
        _NC_CACHE = build_bass()
    return _NC_CACHE


def _axon_runner():
    """Jit the SPMD NEFF exec once (no donation; kernel writes every output
    element, so reusing non-donated zero buffers across calls is safe)."""
    global _RUNNER_CACHE
    if _RUNNER_CACHE is not None:
        return _RUNNER_CACHE
    import jax
    from jax.experimental.shard_map import shard_map
    from jax.sharding import Mesh, PartitionSpec
    from concourse.bass2jax import (_bass_exec_p, install_neuronx_cc_hook,
                                    partition_id_tensor)

    nc = _get_nc()
    install_neuronx_cc_hook()
    pname = nc.partition_id_tensor.name if nc.partition_id_tensor else None
    in_names, out_names, out_avals = [], [], []
    for alloc in nc.m.functions[0].allocations:
        if not isinstance(alloc, mybir.MemoryLocationSet):
            continue
        name = alloc.memorylocations[0].name
        if alloc.kind == "ExternalInput":
            if name != pname:
                in_names.append(name)
        elif alloc.kind == "ExternalOutput":
            out_names.append(name)
            out_avals.append(jax.core.ShapedArray(
                tuple(alloc.tensor_shape), mybir.dt.np(alloc.dtype)))
    n_params = len(in_names)
    all_names = in_names + out_names
    if pname is not None:
        all_names = all_names + [pname]

    def _body(*args):
        operands = list(args)
        if pname is not None:
            operands.append(partition_id_tensor())
        outs = _bass_exec_p.bind(
            *operands, out_avals=tuple(out_avals), in_names=tuple(all_names),
            out_names=tuple(out_names), lowering_input_output_aliases=(),
            sim_require_finite=True, sim_require_nnan=True, nc=nc)
        return tuple(outs)

    mesh = Mesh(np.asarray(jax.devices()[:N_CORES]), ("core",))
    nin = n_params + len(out_names)
    sharded = jax.jit(
        shard_map(_body, mesh=mesh,
                  in_specs=(PartitionSpec("core"),) * nin,
                  out_specs=(PartitionSpec("core"),) * len(out_names),
                  check_rep=False),
        keep_unused=True)
    zeros = [np.zeros((N_CORES * a.shape[0], *a.shape[1:]), a.dtype)
             for a in out_avals]
    _RUNNER_CACHE = (sharded, in_names, out_names, out_avals, zeros)
    return _RUNNER_CACHE


def _run_axon(in_maps):
    import jax
    sharded, in_names, out_names, out_avals, zeros = _axon_runner()
    concat_in = [
        np.concatenate([np.asarray(in_maps[c][n]) for c in range(N_CORES)],
                       axis=0)
        for n in in_names
    ]
    outs = sharded(*concat_in, *zeros)
    return [
        {n: np.asarray(outs[i]).reshape(N_CORES, *out_avals[i].shape)[c]
         for i, n in enumerate(out_names)}
        for c in range(N_CORES)
    ]


def run(inputs, **kw):
    """Returns (full_output, per-core results list)."""
    from concourse._compat import axon_active

    inputs = {k: np.asarray(v) for k, v in inputs.items()}
    in_maps = shard_inputs(**inputs)
    if axon_active():
        results = _run_axon(in_maps)
    else:
        results = run_bass_kernel_spmd(
            _get_nc(), in_maps, core_ids=list(range(N_CORES)), **kw).results
    B = 2
    out = np.zeros((B, S, D), np.float32)
    for c in range(N_CORES):
        b_idx = c // (N_CORES // 2)
        out[b_idx] += np.asarray(results[c]["outT"]).astype(np.float32).T
    return out, results


def kernel(**inputs):
    out, _ = run(inputs)
    return out



# revision 6
# speedup vs baseline: 1.1426x; 1.1426x over previous
"""Multi-head attention (B=2, S=2048, D=1024, H=16, dk=64) on 8 trn2 cores.

Sharding: batch (2) x head-group (4 heads each) = 8 shards.
Core c handles batch b = c // 4, heads g = c % 4 (heads 4g..4g+3).

Host-side prep per core (unchanged from v1):
  - inputs transposed to [d, s] so the contraction dim lands on SBUF
    partitions with no on-chip transposes,
  - Wq/Wk/Wv column-sharded per head group (1/sqrt(dk) folded into Wq/bq),
  - Wo row-sharded, transposed to [c, j],
  - each core emits a transposed partial output [1024, 2048]; host sums the
    4 partials per batch (bo/4 folded into each partial) and transposes back.

On-chip dataflow v2 (per core) - differences vs v1:
  - attn@V runs with au (=exp scores) as the STATIONARY operand and V as the
    moving operand: out[q, dv'] accumulates over k-tiles.  Matmul cost on the
    PE is (moving columns) per instruction, so this costs 65 cols per k-tile
    instead of 512, cutting attn@V PE time in half vs v1.
  - the denominator lands in column 64 of the same [q, 65] PSUM tile (ones
    column in V'), so softmax normalization is a per-partition reciprocal +
    tensor_scalar multiply on the DVE - the v1 PE-broadcast contraption
    (K=1 matmuls of a hi/lo bf16 split) is gone.
  - attention output is produced in [q, c] layout; a cheap PE transpose
    (identity matmul) flips it to [c, q] for the output projection.
  - instruction emission interleaves projection / attn@V / transpose /
    out-proj matmuls between score-matmul rounds so the PE and Act engines
    (exp is ~133us of Act time) both stay busy; score PSUM rotates through
    2 double-buffered 2-slot tiles, attn@V and general work each get 2
    more PSUM banks (8 banks exactly).

All matmuls run in bf16 (fp32 PSUM accumulate).  Full-kernel relative error
vs the fp32 reference is ~6e-3.
"""

from collections import deque
from contextlib import ExitStack

import ml_dtypes
import numpy as np

import concourse.bacc as bacc
import concourse.mybir as mybir
import concourse.tile as tile
from concourse.bass_utils import run_bass_kernel_spmd

F32 = mybir.dt.float32
BF16 = mybir.dt.bfloat16

D = 1024          # d_model
S = 2048          # sequence length
HCORE = 4         # heads per core
DK = 64           # head dim
M = HCORE * DK    # 256 sharded projection width
P = 128

N_CORES = 8
ST = 512          # s-tile (free dim of projection / q-tile)
N_ST = S // ST    # 4
N_DT = D // P     # 8 contraction tiles for projections
N_KT = S // P     # 16 k-tiles for attention
N_JT = D // P     # 8 output row tiles
N_PAIR = N_ST * HCORE   # 16 (qt, h) pairs
N_RND = 8         # score rounds per pair, 2 k-tiles each


def build_mha_tile(tc, outs, ins):
    nc = tc.nc
    xqT, xkT, xvT = ins["xqT"], ins["xkT"], ins["xvT"]
    wq, wk, wv, woT = ins["wq"], ins["wk"], ins["wv"], ins["woT"]
    bq, bk, bvb, bo4 = ins["bq"], ins["bk"], ins["bvb"], ins["bo4"]
    ident = ins["ident"]
    outT = outs["outT"]

    ctx = ExitStack()
    ec = ctx.enter_context
    const = ec(tc.tile_pool(name="const", bufs=1))
    persist = ec(tc.tile_pool(name="persist", bufs=1))
    xin_pool = ec(tc.tile_pool(name="xin", bufs=6))
    au_pool = ec(tc.tile_pool(name="au", bufs=18))
    ob_pool = ec(tc.tile_pool(name="outb", bufs=2))
    small = ec(tc.tile_pool(name="small", bufs=4))
    sc_ps = ec(tc.tile_pool(name="sc_ps", bufs=2, space="PSUM"))
    av_ps = ec(tc.tile_pool(name="av_ps", bufs=2, space="PSUM"))
    pp_ps = ec(tc.tile_pool(name="pp_ps", bufs=2, space="PSUM"))

    xqT3 = xqT.rearrange("(dt p) s -> p dt s", p=P)
    xkT3 = xkT.rearrange("(dt p) s -> p dt s", p=P)
    xvT3 = xvT.rearrange("(dt p) s -> p dt s", p=P)
    outT3 = outT.rearrange("(jt p) s -> p jt s", p=P)

    # ---- persistent activations ----
    QT_sb = persist.tile([P, 2, S], BF16)          # [p, mt, s]
    KT_sb = persist.tile([P, 2, S], BF16)
    V_sb = persist.tile([P, N_KT, HCORE, DK + 1], BF16)   # [p, kt, h, dv']
    cat_sb = persist.tile([P, N_KT, M], BF16)      # attn out [q-part, qtile, c]
    catT_sb = persist.tile([P, 2, S], BF16)        # transposed [c-part, ct, q]

    nc.vector.memset(V_sb[:, :, :, DK], 1.0)      # ones column for denominators

    # The `warm` DVE touches absorb each bias tile's DMA-lane wait so it
    # never lands as a 2nd sync wait on a hot DVE op (walrus allows only 1).
    warm = const.tile([P, 16], F32)

    # ---- const + input stream DMAs (issued up front, in consumption order;
    # the 6-buf xin ring makes later input DMAs wait for frees, which all
    # happen well before the first output DMA needs the queue) ----
    wk_sb = const.tile_from(wk)
    bk_sb = const.tile_from(bk)
    nc.vector.tensor_copy(warm[:, 2:4], bk_sb)
    xin = {}

    def load_x(key, src3, st):
        t = xin_pool.tile([P, N_DT, ST], BF16, tag="xt")
        nc.sync.dma_start(t, src3[:, :, st * ST:(st + 1) * ST])
        xin[(key, st)] = t

    load_x("k", xkT3, 0)
    load_x("k", xkT3, 1)
    wq_sb = const.tile_from(wq)
    bq_sb = const.tile_from(bq)
    nc.vector.tensor_copy(warm[:, 0:2], bq_sb)
    load_x("q", xqT3, 0)
    load_x("k", xkT3, 2)
    load_x("k", xkT3, 3)
    wv_sb = const.tile_from(wv)
    bvb_sb = const.tile_from(bvb)
    nc.vector.tensor_copy(warm[:, 12:16], bvb_sb[:, 0:4])
    for st in range(N_ST):
        load_x("v", xvT3, st)
    for st in range(1, N_ST):
        load_x("q", xqT3, st)
    woT_sb = const.tile_from(woT)
    bo4_sb = const.tile_from(bo4)
    nc.vector.tensor_copy(warm[:, 4:12], bo4_sb)
    id_sb = const.tile_from(ident)

    # ---- emission helpers ----
    def proj_qk_mt(key, w_sb, b_sb, dst_sb, st, mt):
        """dst[:, mt, st-slice] = w.T @ xT + b for one 128-partition tile."""
        xt = xin[(key, st)]
        ps = pp_ps.tile([P, ST], F32, tag="pp", name="qk_ps")
        for dt in range(N_DT):
            nc.tensor.matmul(
                ps,
                w_sb[:, dt, mt * P:(mt + 1) * P],
                xt[:, dt, :],
                start=(dt == 0), stop=(dt == N_DT - 1))
        nc.vector.tensor_scalar_add(
            dst_sb[:, mt, st * ST:(st + 1) * ST], ps, b_sb[:, mt:mt + 1])

    def v_chunk(kt):
        """V[kt-rows, h, dv] = xvT[:, kt].T @ wv + bv."""
        xt = xin[("v", kt // 4)]
        kl = kt % 4
        ps = pp_ps.tile([P, ST], F32, tag="pp", name="v_ps")[:, :M]
        for dt in range(N_DT):
            nc.tensor.matmul(
                ps, xt[:, dt, kl * P:(kl + 1) * P], wv_sb[:, dt, :],
                start=(dt == 0), stop=(dt == N_DT - 1))
        nc.vector.tensor_add(
            out=V_sb[:, kt, :, 0:DK],
            in0=ps.rearrange("p (h d) -> p h d", h=HCORE),
            in1=bvb_sb.rearrange("p (h d) -> p h d", h=HCORE))

    def transp_chunk(qt, ct):
        """catT[:, ct, qt-block] = cat[qt-block, ct-block].T via PE."""
        for qtg in range(4):
            qq = qt * 4 + qtg
            tp = pp_ps.tile([P, 1024], BF16, tag="pp", name="tp_ps")[:, :P]
            nc.tensor.transpose(
                tp, cat_sb[:, qq, ct * P:(ct + 1) * P], id_sb)
            nc.vector.tensor_copy(
                catT_sb[:, ct, qq * P:(qq + 1) * P], tp)

    ob_tiles = {}

    def outp_chunk(qt, jt):
        """outT[jt, qt-slice] = woT.T @ catT + bo/4, DMA at jt 3 and 7."""
        qs = slice(qt * ST, (qt + 1) * ST)
        if jt == 0:
            ob_tiles[qt] = ob_pool.tile([P, N_JT, ST], BF16, tag="ob",
                                        name="ob")
        ob = ob_tiles[qt]
        ps = pp_ps.tile([P, ST], F32, tag="pp", name="op_ps")
        for ct in range(2):
            nc.tensor.matmul(
                ps,
                woT_sb[:, ct, jt * P:(jt + 1) * P],
                catT_sb[:, ct, qs],
                start=(ct == 0), stop=(ct == 1))
        nc.vector.tensor_scalar_add(ob[:, jt, :], ps, bo4_sb[:, jt:jt + 1])
        if jt == N_JT // 2 - 1:
            nc.sync.dma_start(outT3[:, :N_JT // 2, qs], ob[:, :N_JT // 2, :])
        elif jt == N_JT - 1:
            nc.sync.dma_start(outT3[:, N_JT // 2:, qs], ob[:, N_JT // 2:, :])

    # ---- filler scheduler ----
    fillers = deque()

    def enq(key, cycles, fn):
        fillers.append((key, cycles, fn))

    done_keys = set()

    def run_next():
        key, cy, fn = fillers.popleft()
        fn()
        done_keys.add(key)
        return cy

    def fill(budget):
        while fillers and budget > 0:
            budget -= run_next()

    def drain_until(key):
        if key in done_keys:
            return
        while fillers:
            run_next()
            if key in done_keys:
                return

    for st in range(1, N_ST):
        for mt in range(2):
            enq(("K", st, mt), 4096,
                lambda st=st, mt=mt: proj_qk_mt("k", wk_sb, bk_sb, KT_sb, st, mt))
    for kt in range(N_KT):
        enq(("V", kt), 2048, lambda kt=kt: v_chunk(kt))
    for st in range(1, N_ST):
        for mt in range(2):
            enq(("Q", st, mt), 4096,
                lambda st=st, mt=mt: proj_qk_mt("q", wq_sb, bq_sb, QT_sb, st, mt))

    # ---- attention emission ----
    def sc_round(qt, h, r):
        """Scores for k-tiles (2r, 2r+1) -> exp -> au tile [P, 2, ST]."""
        mt, p0 = h // 2, (h % 2) * DK
        qs = slice(qt * ST, (qt + 1) * ST)
        sc = sc_ps.tile([P, 2, ST], F32, tag="sc")
        au = au_pool.tile([P, 2, ST], BF16, tag="au")
        for i in range(2):
            kt = 2 * r + i
            nc.tensor.matmul(
                sc[:, i, :],
                KT_sb[p0:p0 + DK, mt, kt * P:(kt + 1) * P],
                QT_sb[p0:p0 + DK, mt, qs],
                start=True, stop=True)
        nc.scalar.activation(au, sc, mybir.ActivationFunctionType.Exp)
        return au

    def attnv_half(qt, h, au_tiles, qtg, half):
        """Half an attn@V chain: psum[q, dv'] over k-tiles half*8..half*8+7."""
        if half == 0:
            ps = av_ps.tile([P, ST], F32, tag="av", name="av_ps")[:, :DK + 1]
            av_open[(qtg,)] = ps
        else:
            ps = av_open.pop((qtg,))
        q0 = qtg * P
        for kl in range(8):
            kt = half * 8 + kl
            nc.tensor.matmul(
                ps,
                au_tiles[kt // 2][:, kt % 2, q0:q0 + P],
                V_sb[:, kt, h, :],
                start=(kt == 0), stop=(kt == N_KT - 1))
        if half == 1:
            recip = small.tile([P, 1], F32, tag="recip")
            nc.vector.reciprocal(recip, ps[:, DK:DK + 1])
            nc.vector.tensor_scalar_mul(
                cat_sb[:, qt * 4 + qtg, h * DK:(h + 1) * DK],
                ps[:, 0:DK], recip)

    av_open = {}

    # pre-pair work: K st0 + Q st0 so pair-0 scores can start early
    for mt in range(2):
        proj_qk_mt("k", wk_sb, bk_sb, KT_sb, 0, mt)
    for mt in range(2):
        proj_qk_mt("q", wq_sb, bq_sb, QT_sb, 0, mt)

    prev = None       # (qt, h, au_tiles) of previous pair
    for p in range(N_PAIR):
        qt, h = divmod(p, HCORE)
        if h == 0 and qt > 0:
            drain_until(("Q", qt, 1))
        au_tiles = []
        for r in range(N_RND):
            if qt == 0 and h == 0 and r >= 2 and r % 2 == 0:
                drain_until(("K", r // 2, 1))
            au_tiles.append(sc_round(qt, h, r))
            if prev is not None:
                pqt, ph, pau = prev
                if p == 1:
                    drain_until(("V", 7 if r % 2 == 0 else 15))
                attnv_half(pqt, ph, pau, r // 2, r % 2)
                fill(900)
            else:
                fill(2400)
        prev = (qt, h, au_tiles)
        # attn@V of pair p-1 is now fully emitted (it was pair p-2's au);
        # after finishing pair p's rounds, pair p-1 == prev-before-loop ...
        # bookkeeping below uses the pair whose attn@V just completed: p-1.
        if p >= 1:
            pqt, ph = divmod(p - 1, HCORE)
            if ph == 1:
                enq(("T", pqt, 0), 512, lambda pqt=pqt: transp_chunk(pqt, 0))
            if ph == 3:
                enq(("T", pqt, 1), 512, lambda pqt=pqt: transp_chunk(pqt, 1))
                for jt in range(N_JT):
                    enq(("O", pqt, jt), 1024,
                        lambda pqt=pqt, jt=jt: outp_chunk(pqt, jt))

    # ---- tail: attn@V of the final pair, then remaining out-proj ----
    pqt, ph, pau = prev
    for qtg in range(4):
        for half in range(2):
            attnv_half(pqt, ph, pau, qtg, half)
        fill(1100)
    enq(("T", pqt, 1), 512, lambda pqt=pqt: transp_chunk(pqt, 1))
    for jt in range(N_JT):
        enq(("O", pqt, jt), 1024, lambda pqt=pqt, jt=jt: outp_chunk(pqt, jt))
    while fillers:
        run_next()
    ctx.close()


def build_bass():
    nc = bacc.Bacc(trn_type="TRN2", target_bir_lowering=False, debug=False)
    ins = {
        "xqT": nc.dram_tensor("xqT", (D, S), BF16, kind="ExternalInput").ap(),
        "xkT": nc.dram_tensor("xkT", (D, S), BF16, kind="ExternalInput").ap(),
        "xvT": nc.dram_tensor("xvT", (D, S), BF16, kind="ExternalInput").ap(),
        "wq": nc.dram_tensor("wq", (P, N_DT, M), BF16, kind="ExternalInput").ap(),
        "wk": nc.dram_tensor("wk", (P, N_DT, M), BF16, kind="ExternalInput").ap(),
        "wv": nc.dram_tensor("wv", (P, N_DT, M), BF16, kind="ExternalInput").ap(),
        "woT": nc.dram_tensor("woT", (P, 2, D), BF16, kind="ExternalInput").ap(),
        "bq": nc.dram_tensor("bq", (P, 2), F32, kind="ExternalInput").ap(),
        "bk": nc.dram_tensor("bk", (P, 2), F32, kind="ExternalInput").ap(),
        "bvb": nc.dram_tensor("bvb", (P, M), F32, kind="ExternalInput").ap(),
        "bo4": nc.dram_tensor("bo4", (P, N_JT), F32, kind="ExternalInput").ap(),
        "ident": nc.dram_tensor("ident", (P, P), BF16, kind="ExternalInput").ap(),
    }
    outs = {
        "outT": nc.dram_tensor("outT", (D, S), BF16, kind="ExternalOutput").ap(),
    }
    with tile.TileContext(nc) as tc:
        build_mha_tile(tc, outs, ins)
    nc.compile()
    return nc


def shard_inputs(query, key, value, Wq, bq, Wk, bk, Wv, bv, Wo, bo):
    """Build the 8 per-core input maps (all host-side numpy layout prep)."""
    def prep_w(W, ms, scale=1.0):
        # [d, m] -> [p, dt, m]
        wT = (np.asarray(W)[ms, :].T * scale).astype(ml_dtypes.bfloat16)
        return np.ascontiguousarray(
            wT.reshape(N_DT, P, M).transpose(1, 0, 2))

    def prep_b(b, ms, scale=1.0):
        return np.ascontiguousarray(
            (np.asarray(b)[ms] * scale).astype(np.float32).reshape(2, P).T)

    ident = np.eye(P, dtype=ml_dtypes.bfloat16)
    in_maps = []
    for c in range(N_CORES):
        b_idx, g = divmod(c, N_CORES // 2)
        ms = slice(g * M, (g + 1) * M)
        woT = np.ascontiguousarray(Wo[:, ms].T.astype(np.float32))
        in_maps.append({
            "xqT": np.ascontiguousarray(query[b_idx].T.astype(ml_dtypes.bfloat16)),
            "xkT": np.ascontiguousarray(key[b_idx].T.astype(ml_dtypes.bfloat16)),
            "xvT": np.ascontiguousarray(value[b_idx].T.astype(ml_dtypes.bfloat16)),
            "wq": prep_w(Wq, ms, 1.0 / np.sqrt(DK)),
            "wk": prep_w(Wk, ms),
            "wv": prep_w(Wv, ms),
            "woT": np.ascontiguousarray(
                woT.astype(ml_dtypes.bfloat16).reshape(2, P, D).transpose(1, 0, 2)),
            "bq": prep_b(bq, ms, 1.0 / np.sqrt(DK)),
            "bk": prep_b(bk, ms),
            "bvb": np.ascontiguousarray(
                np.tile(np.asarray(bv)[ms].astype(np.float32), (P, 1))),
            "bo4": np.ascontiguousarray(
                (np.asarray(bo) / (N_CORES // 2)).astype(np.float32)
                .reshape(N_JT, P).T),
            "ident": ident,
        })
    return in_maps


_NC_CACHE = None
_RUNNER_CACHE = None


def _get_nc():
    global _NC_CACHE
    if _NC_CACHE is None:
        _NC_CACHE = build_bass()
    return _NC_CACHE


def _axon_runner():
    """Jit the SPMD NEFF exec once (no donation; kernel writes every output
    element, so reusing non-donated zero buffers across calls is safe)."""
    global _RUNNER_CACHE
    if _RUNNER_CACHE is not None:
        return _RUNNER_CACHE
    import jax
    from jax.experimental.shard_map import shard_map
    from jax.sharding import Mesh, PartitionSpec
    from concourse.bass2jax import (_bass_exec_p, install_neuronx_cc_hook,
                                    partition_id_tensor)

    nc = _get_nc()
    install_neuronx_cc_hook()
    pname = nc.partition_id_tensor.name if nc.partition_id_tensor else None
    in_names, out_names, out_avals = [], [], []
    for alloc in nc.m.functions[0].allocations:
        if not isinstance(alloc, mybir.MemoryLocationSet):
            continue
        name = alloc.memorylocations[0].name
        if alloc.kind == "ExternalInput":
            if name != pname:
                in_names.append(name)
        elif alloc.kind == "ExternalOutput":
            out_names.append(name)
            out_avals.append(jax.core.ShapedArray(
                tuple(alloc.tensor_shape), mybir.dt.np(alloc.dtype)))
    n_params = len(in_names)
    all_names = in_names + out_names
    if pname is not None:
        all_names = all_names + [pname]

    def _body(*args):
        operands = list(args)
        if pname is not None:
            operands.append(partition_id_tensor())
        outs = _bass_exec_p.bind(
            *operands, out_avals=tuple(out_avals), in_names=tuple(all_names),
            out_names=tuple(out_names), lowering_input_output_aliases=(),
            sim_require_finite=True, sim_require_nnan=True, nc=nc)
        return tuple(outs)

    mesh = Mesh(np.asarray(jax.devices()[:N_CORES]), ("core",))
    nin = n_params + len(out_names)
    sharded = jax.jit(
        shard_map(_body, mesh=mesh,
                  in_specs=(PartitionSpec("core"),) * nin,
                  out_specs=(PartitionSpec("core"),) * len(out_names),
                  check_rep=False),
        keep_unused=True)
    zeros = [np.zeros((N_CORES * a.shape[0], *a.shape[1:]), a.dtype)
             for a in out_avals]
    _RUNNER_CACHE = (sharded, in_names, out_names, out_avals, zeros)
    return _RUNNER_CACHE


def _run_axon(in_maps):
    import jax
    sharded, in_names, out_names, out_avals, zeros = _axon_runner()
    concat_in = [
        np.concatenate([np.asarray(in_maps[c][n]) for c in range(N_CORES)],
                       axis=0)
        for n in in_names
    ]
    outs = sharded(*concat_in, *zeros)
    return [
        {n: np.asarray(outs[i]).reshape(N_CORES, *out_avals[i].shape)[c]
         for i, n in enumerate(out_names)}
        for c in range(N_CORES)
    ]


def run(inputs, **kw):
    """Returns (full_output, per-core results list)."""
    from concourse._compat import axon_active

    inputs = {k: np.asarray(v) for k, v in inputs.items()}
    in_maps = shard_inputs(**inputs)
    if axon_active():
        results = _run_axon(in_maps)
    else:
        results = run_bass_kernel_spmd(
            _get_nc(), in_maps, core_ids=list(range(N_CORES)), **kw).results
    B = 2
    out = np.zeros((B, S, D), np.float32)
    for c in range(N_CORES):
        b_idx = c // (N_CORES // 2)
        out[b_idx] += np.asarray(results[c]["outT"]).astype(np.float32).T
    return out, results


def kernel(**inputs):
    out, _ = run(inputs)
    return out
